# revision 39
# baseline (speedup 1.0000x reference)
"""BigBird attention on 8 Trainium2 NeuronCores.

Sharding: cores 0-3 take batch 0, cores 4-7 batch 1; each core computes 3 of
the 12 heads end-to-end (q/k/v projection, masked dense attention, its slice
of the output projection). Host work is limited to input transposes/slices
and the final 4-way partial-sum (f16 partials) + output bias.

Per-core dataflow (all matmuls bf16/f32r with f32 PSUM accumulation):
  - Scores are transposed, sT[j, i] = k_j . q_i, contraction depth 64. Heads
    0 and 1 live on partitions 0-63 / 64-127 of qT/kT, so their score
    matmuls occupy disjoint PE row-groups (row tiling); head 2 is duplicated
    into partitions 64-127 and paired across key tiles. Each pair lands in
    one (128, 1024) 2-bank PSUM tile -> one exp activation covers both.
  - exp on ScalarE (scale folded); the mask is applied as ONE bf16
    tensor_tensor multiply per unit (stride-0 broadcast of the key-tile mask
    for phase-1 units, contiguous key-tile-pair slice for phase-2), mostly
    on VectorE with GpSimd taking every 6th unit.
  - AV accumulates [v | 1] so the softmax denominator rides along as row 64.
  - Emission is software-pipelined: AV of unit u is emitted after scores of
    unit u+2 (psA bufs=2), with the previous stripe's normalize + output
    projection spread through this stripe's units.
  - Output projection is head-stacked: heads 0+1 occupy partitions 0-127 of
    one osb tile so their Wo contribution is a single contraction-128
    matmul; head 2 accumulates on top (4 matmuls per 128-token block).
  - The last stripe's h0/h1 normalizes interleave into its phase-2 units so
    the drain is only: h2 normalize + 4 output-projection blocks, with the
    final y DMAs spread across four queues.
"""

import sys

sys.path.insert(0, "/opt/trn_rl_repo")

import numpy as np
import ml_dtypes

import concourse.bass as bass
import concourse.tile as tile
from concourse import bacc
from concourse import mybir
from concourse.bass_utils import run_bass_kernel_spmd

B, T, D, H, HD = 2, 2048, 768, 12, 64
NCORES = 8
HPC = 3  # heads per core
DPC = HPC * HD  # 192 projected dims per core
NKT = D // 128  # 6 contraction tiles (biases handled separately)
SCALE = HD ** -0.5
IT = 512  # query stripe
NIT = T // IT
JT = 128  # key tile (partition dim of transposed scores)
NJT = T // JT

F32 = mybir.dt.float32
F32R = mybir.dt.float32r
BF16 = mybir.dt.bfloat16
F16 = mybir.dt.float16

LAST_RESULTS = None  # BassKernelResults of the most recent run (for test.py)

_NC = None


def _build_nc():
    nc = bacc.Bacc(None, target_bir_lowering=False)

    xT_b = nc.declare_dram_parameter("xT_b", (D, T), BF16, isOutput=False)
    wq = nc.declare_dram_parameter("wq", (D, DPC), BF16, isOutput=False)
    wk = nc.declare_dram_parameter("wk", (D, DPC), BF16, isOutput=False)
    wqk_hi = nc.declare_dram_parameter("wqk_hi", (D, 128), BF16, isOutput=False)
    bqk = nc.declare_dram_parameter("bqk", (3, 128), F32, isOutput=False)
    wv = nc.declare_dram_parameter("wv", (D, DPC), BF16, isOutput=False)
    bv = nc.declare_dram_parameter("bv", (1, DPC), BF16, isOutput=False)
    woT = nc.declare_dram_parameter("woT", (DPC, D), BF16, isOutput=False)
    maskT = nc.declare_dram_parameter("maskT", (T, T), BF16, isOutput=False)
    y = nc.declare_dram_parameter("y", (T, D), F16, isOutput=True)

    with tile.TileContext(nc) as tc:
        _emit(nc, tc, xT_b, wq, wk, wqk_hi, bqk, wv, bv, woT, maskT, y)
    nc.finalize()
    return nc


def _emit(nc, tc, xT_b, wq, wk, wqk_hi, bqk, wv, bv, woT, maskT, y):
    import contextlib

    ctx = contextlib.ExitStack()
    with ctx:
        res = ctx.enter_context(tc.tile_pool(name="res", bufs=1))  # residents
        mpool = ctx.enter_context(tc.tile_pool(name="mask", bufs=2))
        epool = ctx.enter_context(tc.tile_pool(name="e", bufs=6))
        empool = ctx.enter_context(tc.tile_pool(name="em", bufs=6))
        rawpool = ctx.enter_context(tc.tile_pool(name="rawp", bufs=4))
        dnpool = ctx.enter_context(tc.tile_pool(name="dnp", bufs=4))
        osbpool = ctx.enter_context(tc.tile_pool(name="osbp", bufs=6))
        small = ctx.enter_context(tc.tile_pool(name="small", bufs=3))
        ypool = ctx.enter_context(tc.tile_pool(name="ysb", bufs=3))

        psA = ctx.enter_context(tc.tile_pool(name="psA", bufs=2, space="PSUM"))
        psO = ctx.enter_context(tc.tile_pool(name="psO", bufs=2, space="PSUM"))
        psW = ctx.enter_context(tc.tile_pool(name="psW", bufs=1, space="PSUM"))

        # ---- resident loads, spread across the four DMA-capable queues -----
        # Priority order: what the first projection pass (q nt0, k nt0: x
        # halves 0 + wq + wk) needs goes first; everything else follows.
        xk = [res.tile([128, T], BF16, name=f"xk{kt}") for kt in range(NKT)]
        wq_sb = res.tile([128, NKT, DPC], BF16, name="wq_sb")
        wk_sb = res.tile([128, NKT, DPC], BF16, name="wk_sb")
        wv_sb = res.tile([128, NKT, DPC], BF16, name="wv_sb")
        wqkhi_sb = res.tile([128, NKT, 128], BF16, name="wqkhi_sb")

        def kt_slice(dram, kt):
            return dram[kt * 128 : (kt + 1) * 128, :]

        bqk_sb = res.tile([128, 3], F32, name="bqk_sb")
        nc.sync.dma_start(out=bqk_sb, in_=bqk.rearrange("a p -> p a"))
        bv_sb = res.tile([1, DPC], BF16, name="bv_sb")

        loads = []
        for kt in range(NKT):
            loads += [
                (wq_sb[:, kt, :], kt_slice(wq, kt)),
                (xk[kt][:, 0:512], kt_slice(xT_b, kt)[:, 0:512]),
                (wk_sb[:, kt, :], kt_slice(wk, kt)),
            ]
        for kt in range(NKT):
            loads += [
                (wqkhi_sb[:, kt, :], kt_slice(wqk_hi, kt)),
                (xk[kt][:, 512:1024], kt_slice(xT_b, kt)[:, 512:1024]),
            ]
        for kt in range(NKT):
            loads += [
                (xk[kt][:, 1024:1536], kt_slice(xT_b, kt)[:, 1024:1536]),
                (wv_sb[:, kt, :], kt_slice(wv, kt)),
            ]
        for kt in range(NKT):
            loads.append((xk[kt][:, 1536:2048], kt_slice(xT_b, kt)[:, 1536:2048]))
        loads.append((bv_sb, bv[0:1, :]))
        # sync/gpsimd take most traffic (cheap issue); scalar helps at
        # startup (its engine is idle until the first projection lands)
        qpat = [nc.sync, nc.gpsimd, nc.scalar]
        for i, (dst, src) in enumerate(loads):
            qpat[i % len(qpat)].dma_start(out=dst, in_=src)

        # head-stacked Wo slices: heads 0+1 contiguous on partitions 0-127
        # (one contraction-128 matmul), head 2 separate on partitions 0-63.
        woT01_sb = res.tile([128, D], BF16, name="woT01_sb")
        nc.sync.dma_start(out=woT01_sb, in_=woT[0:128, :])
        woT2_sb = res.tile([64, D], BF16, name="woT2_sb")
        nc.gpsimd.dma_start(out=woT2_sb, in_=woT[128:192, :])

        ones_row = res.tile([1, T], BF16, name="ones_row")
        nc.vector.memset(ones_row, 1.0)

        ones_col = res.tile([1, HD], BF16)
        nc.vector.memset(ones_col, 1.0)

        # per-stripe mask mega-tile (128, NJT, 512): one paired DMA per two
        # adjacent key tiles (256 DRAM rows -> (128, 2, 512) strided)
        m_stripes = {}  # ith -> (128, NJT, 512) tile

        def prefetch_masks(ith, queues=(None,)):
            if ith >= NIT:
                return
            isl = slice(ith * IT, (ith + 1) * IT)
            m = mpool.tile([JT, NJT, IT], BF16, tag="mask", name="m_stripe")
            for jp in range(NJT // 2):
                src = maskT[2 * jp * JT : (2 * jp + 2) * JT, isl].rearrange(
                    "(a p) q -> p a q", p=JT
                )
                q = queues[jp % len(queues)] or nc.sync
                q.dma_start(out=m[:, 2 * jp : 2 * jp + 2, :], in_=src)
            m_stripes[ith] = m

        # stripe-0 masks load during the projections, on the two cheapest
        # queues; later stripes prefetch on sync alone.
        prefetch_masks(0, queues=(nc.sync, nc.gpsimd))

        # ---- stage A: projections ------------------------------------------
        qT_a = res.tile([128, T], BF16)  # q heads 0,1 (dims on partitions)
        kT_a = res.tile([128, T], BF16)
        qT_b = res.tile([128, T], BF16)  # q head 2 + duplicate in rows 64-127
        kT_b = res.tile([128, T], BF16)

        def proj_pass(w_sb, nt, emit_out):
            ns = slice(nt * 1024, (nt + 1) * 1024)
            ps = psA.tile([128, 1024], F32, tag="psA", name="psqk")
            for half in range(2):
                fs = slice(nt * 1024 + half * 512, nt * 1024 + half * 512 + 512)
                hs = slice(half * 512, (half + 1) * 512)
                for kt in range(NKT):
                    nc.tensor.matmul(
                        out=ps[:, hs],
                        lhsT=w_sb[:, kt, 0:128],
                        rhs=xk[kt][:, fs],
                        start=(kt == 0),
                        stop=(kt == NKT - 1),
                    )
            emit_out(ps, ns)

        def emit_q(ps, ns):
            nc.scalar.add(out=qT_a[:, ns], in_=ps, add=bqk_sb[:, 0:1])

        def emit_k(ps, ns):
            nc.scalar.add(out=kT_a[:, ns], in_=ps, add=bqk_sb[:, 1:2])

        def emit_hi(ps, ns):
            nc.scalar.add(out=qT_b[0:64, ns], in_=ps[0:64, :], add=bqk_sb[0:64, 2:3])
            nc.scalar.add(
                out=kT_b[0:64, ns], in_=ps[64:128, :], add=bqk_sb[64:128, 2:3]
            )

        # nt=0 passes first (they only need the x column-halves loaded
        # first), then nt=1; PE never waits on the tail of the x load.
        proj_pass(wq_sb, 0, emit_q)
        proj_pass(wk_sb, 0, emit_k)
        proj_pass(wqkhi_sb, 0, emit_hi)
        proj_pass(wq_sb, 1, emit_q)
        proj_pass(wk_sb, 1, emit_k)
        proj_pass(wqkhi_sb, 1, emit_hi)

        # duplicate head 2 q/k into partitions 64-127 (row-group pairing)
        nc.sync.dma_start(out=qT_b[64:128, :], in_=qT_b[0:64, :])
        nc.gpsimd.dma_start(out=kT_b[64:128, :], in_=kT_b[0:64, :])

        # v natural, packed as [v | 1] per head: (128, NJT, HPC, 65) bf16.
        vaug = res.tile([128, NJT, HPC, HD + 1], BF16)
        nc.vector.memset(vaug, 1.0)

        def emit_vproj(jt):
            js = slice(jt * JT, (jt + 1) * JT)
            ps = psW.tile([128, 1024], F32, tag="psW", name="psv")
            for kt in range(NKT):
                nc.tensor.matmul(
                    out=ps[:, 0:DPC],
                    lhsT=xk[kt][:, js],
                    rhs=wv_sb[:, kt, :],
                    start=(kt == 0),
                    stop=False,
                )
            nc.tensor.matmul(
                out=ps[:, 0:DPC], lhsT=ones_row[:, js], rhs=bv_sb,
                start=False, stop=True,
            )
            nc.vector.tensor_copy(
                out=vaug[:, jt, :, 0:HD],
                in_=ps[:, 0:DPC].rearrange("p (h d) -> p h d", h=HPC),
            )

        emit_vproj(0)
        emit_vproj(1)

        # ---- stage B: attention --------------------------------------------
        # Unit list: per query stripe, 16 phase-1 units (heads 0+1, one jt
        # each) then 8 phase-2 units (head 2, a pair of jts). Each unit is
        # one (128, 1024) score tile = two concurrent 512-free matmuls.
        units = []
        for ith in range(NIT):
            for jt in range(NJT):
                units.append(("p1", ith, jt))
            for jp in range(NJT // 2):
                units.append(("p2", ith, 2 * jp, 2 * jp + 1))

        oT = {}  # (ith, h) -> psum tile
        stash = {}  # (ith, h) -> (raw, dn) tiles

        ucount = [0]

        def emit_scores(u):
            isl = slice(u[1] * IT, (u[1] + 1) * IT)
            ps = psA.tile([128, 1024], F32, tag="psA", name="sT")
            eT = epool.tile([JT, 1024], BF16, tag="e", name="eT")
            eTm = empool.tile([JT, 1024], BF16, tag="em", name="eTm")
            m = m_stripes[u[1]]
            if u[0] == "p1":
                jt = u[2]
                js = slice(jt * JT, (jt + 1) * JT)
                # heads 0 (rows 0-63) and 1 (rows 64-127): concurrent
                nc.tensor.matmul(
                    out=ps[:, 0:512], lhsT=kT_a[0:64, js], rhs=qT_a[0:64, isl],
                    start=True, stop=True,
                )
                nc.tensor.matmul(
                    out=ps[:, 512:1024], lhsT=kT_a[64:128, js],
                    rhs=qT_a[64:128, isl], start=True, stop=True,
                )
                # same key-tile mask for both halves: stride-0 broadcast
                m_in = m[:, jt : jt + 1, :].to_broadcast((JT, 2, IT))
            else:
                jt0, jt1 = u[2], u[3]
                js0 = slice(jt0 * JT, (jt0 + 1) * JT)
                js1 = slice(jt1 * JT, (jt1 + 1) * JT)
                # head 2 vs its partition-64 duplicate: concurrent
                nc.tensor.matmul(
                    out=ps[:, 0:512], lhsT=kT_b[0:64, js0], rhs=qT_b[0:64, isl],
                    start=True, stop=True,
                )
                nc.tensor.matmul(
                    out=ps[:, 512:1024], lhsT=kT_b[64:128, js1],
                    rhs=qT_b[64:128, isl], start=True, stop=True,
                )
                m_in = m[:, jt0 : jt0 + 2, :]
                if u[2] == 0:
                    prefetch_masks(u[1] + 1)  # next stripe, during phase 2
            nc.scalar.activation(
                out=eT, in_=ps, func=mybir.ActivationFunctionType.Exp,
                scale=SCALE,
            )
            # mask multiply: every other unit is split half-to-GpSimd (its
            # slower rate is hidden by the AV lag), the rest one Vector op
            if ucount[0] % 2 == 0:
                if u[0] == "p1":
                    m0 = m1 = m[:, u[2], :]
                else:
                    m0, m1 = m[:, u[2], :], m[:, u[3], :]
                nc.vector.tensor_mul(
                    out=eTm[:, 0:512], in0=eT[:, 0:512], in1=m0
                )
                nc.gpsimd.tensor_mul(
                    out=eTm[:, 512:1024], in0=eT[:, 512:1024], in1=m1
                )
            elif u[0] == "p1":
                nc.vector.tensor_mul(
                    out=eTm.rearrange("p (a b) -> p a b", a=2),
                    in0=eT.rearrange("p (a b) -> p a b", a=2),
                    in1=m_in,
                )
            else:
                nc.vector.tensor_mul(
                    out=eTm, in0=eT,
                    in1=m_in.rearrange("p a q -> p (a q)"),
                )
            ucount[0] += 1
            return eTm

        def get_oT(ith, h):
            if (ith, h) not in oT:
                oT[(ith, h)] = psO.tile([128, IT], F32, tag="psO", name="oT")
            return oT[(ith, h)]

        def emit_av_one(u, eTm, h, hs):
            # one AV link: head h of unit u from half hs of its eTm
            jt = u[2] if (u[0] == "p1" or hs.start == 0) else u[3]
            t = get_oT(u[1], h)
            nc.tensor.matmul(
                out=t[0 : HD + 1, :],
                lhsT=vaug[:, jt, h, :],
                rhs=eTm[:, hs],
                start=(jt == 0),
                stop=(jt == NJT - 1),
            )

        H0, H1 = slice(0, 512), slice(512, 1024)

        def emit_av_group(group):
            # h-major across the group so consecutive AV matmuls accumulate
            # into the SAME psum tile (no output-bank alternation)
            if group[0][1][0] == "p1":
                for h, hs in ((0, H0), (1, H1)):
                    for _, u, eTm in group:
                        emit_av_one(u, eTm, h, hs)
            else:
                for _, u, eTm in group:
                    emit_av_one(u, eTm, 2, H0)
                    emit_av_one(u, eTm, 2, H1)

        osb01_all = {}  # ith -> (128, IT) f32r stacked heads 0+1
        osb2_all = {}  # ith -> (64, IT) f32r head 2

        def emit_normalize(ith, h, pool=None):
            # broadcast the denominator to HD partitions on the PE, then
            # reciprocal + scale (both reading across the full partition set)
            raw, dnb = stash.pop((ith, h))
            rb = (pool or psW).tile(
                [128, 1024], F32, tag="psA" if pool is not None else "psW",
                name="rb",
            )
            nc.tensor.matmul(
                out=rb[0:HD, 0:IT], lhsT=ones_col, rhs=dnb,
                start=True, stop=True,
            )
            rcb = small.tile([HD, IT], F32, tag="rcb", name="rcb")
            nc.vector.reciprocal_approx_fast(out=rcb, in_=rb[0:HD, 0:IT])
            if h < 2:
                if ith not in osb01_all:
                    osb01_all[ith] = osbpool.tile(
                        [128, IT], BF16, tag="osb01", name=f"osb01_{ith}"
                    )
                dst = osb01_all[ith][h * HD : (h + 1) * HD, :]
            else:
                osb2_all[ith] = osbpool.tile(
                    [HD, IT], BF16, tag="osb2", name=f"osb2_{ith}"
                )
                dst = osb2_all[ith]
            nc.vector.tensor_mul(out=dst, in0=raw[0:HD, :], in1=rcb)

        def emit_wo_p1(ith, tb, pool=None):
            # heads 0+1 of one 128-token block: one contraction-128 matmul
            # per free chunk into a held psum tile
            ts = slice(tb * 128, (tb + 1) * 128)
            yps = (pool or psW).tile(
                [128, 1024], F32, tag="psA" if pool is not None else "psW",
                name="yps",
            )
            for n0, nsz in ((0, 512), (512, 256)):
                nsl = slice(n0, n0 + nsz)
                nc.tensor.matmul(
                    out=yps[:, nsl], lhsT=osb01_all[ith][:, ts],
                    rhs=woT01_sb[:, nsl], start=True, stop=False,
                )
            return yps

        def emit_wo_p2(ith, tb, yps, yq, cast_eng=None):
            # head 2 accumulates on top, then cast + store
            t0 = ith * IT + tb * 128
            ts = slice(tb * 128, (tb + 1) * 128)
            for n0, nsz in ((0, 512), (512, 256)):
                nsl = slice(n0, n0 + nsz)
                nc.tensor.matmul(
                    out=yps[:, nsl], lhsT=osb2_all[ith][:, ts],
                    rhs=woT2_sb[:, nsl], start=False, stop=True,
                )
            ysb = ypool.tile([128, D], F16, tag="ysb", name="ysb")
            if cast_eng is nc.scalar:
                nc.scalar.copy(out=ysb, in_=yps[:, 0:D])
            else:
                nc.vector.tensor_copy(out=ysb, in_=yps[:, 0:D])
            yq.dma_start(out=y[t0 : t0 + 128, :], in_=ysb)

        def emit_wo(ith, tb, yq, pool=None, cast_eng=None):
            yps = emit_wo_p1(ith, tb, pool=pool)
            emit_wo_p2(ith, tb, yps, yq, cast_eng=cast_eng)

        def emit_stash(u):
            ith = u[1]
            done = []
            if u[0] == "p1" and u[2] == NJT - 1:
                done = [0, 1]
            elif u[0] == "p2" and u[3] == NJT - 1:
                done = [2]
            for h in done:
                t = oT.pop((ith, h))
                dnb = dnpool.tile([1, IT], BF16, tag="dn", name="dnb")
                nc.vector.tensor_copy(out=dnb, in_=t[HD : HD + 1, :])
                raw = rawpool.tile([HD, IT], BF16, tag="raw", name="raw")
                nc.vector.tensor_copy(out=raw, in_=t[0:HD, :])
                stash[(ith, h)] = (raw, dnb)

        # Software-pipelined emission: AV lags scores by 2 units, the
        # previous stripe's normalize + Wo spread through this stripe's
        # units, and the last stripe's h0/h1 normalizes land in its own
        # phase-2 units so the drain is minimal.
        UPS = NJT + NJT // 2  # units per stripe
        LAG = 3

        wo_done = set()

        def do_norm(s, h):
            emit_normalize(s, h)

        wo_pend = {}

        def interleave_tail(idx):
            s, pos = idx // UPS, idx % UPS
            if s < 1:
                return
            if pos == 4:
                do_norm(s - 1, 0)
            elif pos == 7:
                do_norm(s - 1, 1)
            elif pos == 10:
                do_norm(s - 1, 2)
            elif pos in (13, 16, 19, 22):
                tb = (pos - 13) // 3
                wo_done.add((s - 1, tb))
                wo_pend[(s - 1, tb)] = emit_wo_p1(s - 1, tb)
            elif pos in (14, 17, 20, 23):
                tb = (pos - 14) // 3
                emit_wo_p2(s - 1, tb, wo_pend.pop((s - 1, tb)), nc.sync)
            if s == NIT - 1:
                if pos == 15:
                    do_norm(s, 0)
                elif pos == 18:
                    do_norm(s, 1)

        def pop_avs(n):
            group = [pend.pop(0) for _ in range(n)]
            emit_av_group(group)
            for pi, pu, _ in group:
                emit_stash(pu)
                interleave_tail(pi)

        pend = []
        for i, u in enumerate(units):
            eTm = emit_scores(u)
            if i < NJT - 2:
                emit_vproj(i + 2)
            pend.append((i, u, eTm))
            if len(pend) > LAG:
                if pend[0][1][0] == pend[1][1][0]:
                    pop_avs(2)
                else:
                    pop_avs(1)
        while pend:
            n = 2 if len(pend) >= 2 and pend[0][1][0] == pend[1][1][0] else 1
            pop_avs(n)
        # drain: last stripe's h2 normalize + its 4 output-projection
        # blocks, using the now-free psA banks for 2-deep pipelining and
        # the now-idle ScalarE for half the output casts.
        for ith in range(NIT):
            for h in range(HPC):
                if (ith, h) in stash:
                    emit_normalize(ith, h, pool=psA)
        yqs = [nc.sync, nc.gpsimd, nc.scalar]
        ti = 0
        for ith in range(NIT):
            for tb in range(IT // 128):
                if (ith, tb) not in wo_done:
                    emit_wo(
                        ith, tb, yqs[ti % len(yqs)], pool=psA,
                        cast_eng=nc.scalar if ti % 2 == 0 else nc.vector,
                    )
                    ti += 1


def _host_prep(x, Wq, bq, Wk, bk, Wv, bv, Wo, bo, mask):
    """Build the 8 per-core input maps."""
    x = np.asarray(x, dtype=np.float32)
    mask_np = np.asarray(mask)
    maskT_bf = np.ascontiguousarray(mask_np.T).astype(ml_dtypes.bfloat16)

    xTs = [np.ascontiguousarray(x[b].T).astype(ml_dtypes.bfloat16) for b in range(B)]

    def w_cols(W, cols):
        return np.ascontiguousarray(
            np.asarray(W, np.float32).T[:, cols]
        ).astype(ml_dtypes.bfloat16)

    def wo_rows(W, cols):
        return np.ascontiguousarray(
            np.asarray(W, np.float32).T[cols, :]
        ).astype(ml_dtypes.bfloat16)

    in_maps = []
    for core in range(NCORES):
        b = core // 4
        h0 = HPC * (core % 4)
        cols = np.arange(h0 * HD, (h0 + HPC) * HD)
        wq_a = w_cols(Wq, cols)
        wk_a = w_cols(Wk, cols)
        in_maps.append(
            {
                "xT_b": xTs[b],
                "wq": wq_a,
                "wk": wk_a,
                "wqk_hi": np.concatenate(
                    [wq_a[:, 128:192], wk_a[:, 128:192]], axis=1
                ),
                "bqk": np.stack(
                    [
                        np.asarray(bq, np.float32)[cols][0:128],
                        np.asarray(bk, np.float32)[cols][0:128],
                        np.concatenate(
                            [
                                np.asarray(bq, np.float32)[cols][128:192],
                                np.asarray(bk, np.float32)[cols][128:192],
                            ]
                        ),
                    ]
                ),
                "wv": w_cols(Wv, cols),
                "bv": np.asarray(bv, np.float32)[cols][None, :].astype(
                    ml_dtypes.bfloat16
                ),
                "woT": wo_rows(Wo, cols),
                "maskT": maskT_bf,
            }
        )
    return in_maps


def kernel(x, Wq, bq, Wk, bk, Wv, bv, Wo, bo, mask):
    global _NC, LAST_RESULTS
    if _NC is None:
        _NC = _build_nc()

    in_maps = _host_prep(x, Wq, bq, Wk, bk, Wv, bv, Wo, bo, mask)
    res = run_bass_kernel_spmd(_NC, in_maps, list(range(NCORES)))
    LAST_RESULTS = res

    bo = np.asarray(bo, np.float32)
    out = np.zeros((B, T, D), np.float32)
    for core in range(NCORES):
        out[core // 4] += np.asarray(res.results[core]["y"], np.float32)
    out += bo
    return out


# revision 40
# speedup vs baseline: 1.0088x; 1.0088x over previous
"""BigBird attention on 8 Trainium2 NeuronCores.

Sharding: cores 0-3 take batch 0, cores 4-7 batch 1; each core computes 3 of
the 12 heads end-to-end (q/k/v projection, masked dense attention, its slice
of the output projection). Host work is limited to input transposes/slices
and the final 4-way partial-sum (f16 partials) + output bias.

Per-core dataflow (all matmuls bf16/f32r with f32 PSUM accumulation):
  - Scores are transposed, sT[j, i] = k_j . q_i, contraction depth 64. Heads
    0 and 1 live on partitions 0-63 / 64-127 of qT/kT, so their score
    matmuls occupy disjoint PE row-groups (row tiling); head 2 is duplicated
    into partitions 64-127 and paired across key tiles. Each pair lands in
    one (128, 1024) 2-bank PSUM tile -> one exp activation covers both.
  - exp on ScalarE (scale folded); the mask is applied as ONE bf16
    tensor_tensor multiply per unit (stride-0 broadcast of the key-tile mask
    for phase-1 units, contiguous key-tile-pair slice for phase-2), mostly
    on VectorE with GpSimd taking every 6th unit.
  - AV accumulates [v | 1] so the softmax denominator rides along as row 64.
  - Emission is software-pipelined: AV of unit u is emitted after scores of
    unit u+2 (psA bufs=2), with the previous stripe's normalize + output
    projection spread through this stripe's units.
  - Output projection is head-stacked: heads 0+1 occupy partitions 0-127 of
    one osb tile so their Wo contribution is a single contraction-128
    matmul; head 2 accumulates on top (4 matmuls per 128-token block).
  - The last stripe's h0/h1 normalizes interleave into its phase-2 units so
    the drain is only: h2 normalize + 4 output-projection blocks, with the
    final y DMAs spread across four queues.
"""

import sys

sys.path.insert(0, "/opt/trn_rl_repo")

import numpy as np
import ml_dtypes

import concourse.bass as bass
import concourse.tile as tile
from concourse import bacc
from concourse import mybir
from concourse.bass_utils import run_bass_kernel_spmd

B, T, D, H, HD = 2, 2048, 768, 12, 64
NCORES = 8
HPC = 3  # heads per core
DPC = HPC * HD  # 192 projected dims per core
NKT = D // 128  # 6 contraction tiles (biases handled separately)
SCALE = HD ** -0.5
IT = 512  # query stripe
NIT = T // IT
JT = 128  # key tile (partition dim of transposed scores)
NJT = T // JT

F32 = mybir.dt.float32
F32R = mybir.dt.float32r
BF16 = mybir.dt.bfloat16
F16 = mybir.dt.float16

LAST_RESULTS = None  # BassKernelResults of the most recent run (for test.py)

_NC = None


def _build_nc():
    nc = bacc.Bacc(None, target_bir_lowering=False)

    xT_b = nc.declare_dram_parameter("xT_b", (D, T), BF16, isOutput=False)
    wq = nc.declare_dram_parameter("wq", (D, DPC), BF16, isOutput=False)
    wk = nc.declare_dram_parameter("wk", (D, DPC), BF16, isOutput=False)
    wqk_hi = nc.declare_dram_parameter("wqk_hi", (D, 128), BF16, isOutput=False)
    bqk = nc.declare_dram_parameter("bqk", (3, 128), F32, isOutput=False)
    wv = nc.declare_dram_parameter("wv", (D, DPC), BF16, isOutput=False)
    bv = nc.declare_dram_parameter("bv", (1, DPC), BF16, isOutput=False)
    woT = nc.declare_dram_parameter("woT", (DPC, D), BF16, isOutput=False)
    maskT = nc.declare_dram_parameter("maskT", (T, T), BF16, isOutput=False)
    y = nc.declare_dram_parameter("y", (T, D), F16, isOutput=True)

    with tile.TileContext(nc) as tc:
        _emit(nc, tc, xT_b, wq, wk, wqk_hi, bqk, wv, bv, woT, maskT, y)
    nc.finalize()
    return nc


def _emit(nc, tc, xT_b, wq, wk, wqk_hi, bqk, wv, bv, woT, maskT, y):
    import contextlib

    ctx = contextlib.ExitStack()
    with ctx:
        res = ctx.enter_context(tc.tile_pool(name="res", bufs=1))  # residents
        mpool = ctx.enter_context(tc.tile_pool(name="mask", bufs=2))
        epool = ctx.enter_context(tc.tile_pool(name="e", bufs=6))
        empool = ctx.enter_context(tc.tile_pool(name="em", bufs=6))
        rawpool = ctx.enter_context(tc.tile_pool(name="rawp", bufs=4))
        dnpool = ctx.enter_context(tc.tile_pool(name="dnp", bufs=4))
        osbpool = ctx.enter_context(tc.tile_pool(name="osbp", bufs=6))
        small = ctx.enter_context(tc.tile_pool(name="small", bufs=3))
        ypool = ctx.enter_context(tc.tile_pool(name="ysb", bufs=3))

        psA = ctx.enter_context(tc.tile_pool(name="psA", bufs=2, space="PSUM"))
        psO = ctx.enter_context(tc.tile_pool(name="psO", bufs=2, space="PSUM"))
        psW = ctx.enter_context(tc.tile_pool(name="psW", bufs=1, space="PSUM"))

        # ---- resident loads, spread across the four DMA-capable queues -----
        # Priority order: what the first projection pass (q nt0, k nt0: x
        # halves 0 + wq + wk) needs goes first; everything else follows.
        xk = [res.tile([128, T], BF16, name=f"xk{kt}") for kt in range(NKT)]
        wq_sb = res.tile([128, NKT, DPC], BF16, name="wq_sb")
        wk_sb = res.tile([128, NKT, DPC], BF16, name="wk_sb")
        wv_sb = res.tile([128, NKT, DPC], BF16, name="wv_sb")
        wqkhi_sb = res.tile([128, NKT, 128], BF16, name="wqkhi_sb")

        def kt_slice(dram, kt):
            return dram[kt * 128 : (kt + 1) * 128, :]

        bqk_sb = res.tile([128, 3], F32, name="bqk_sb")
        nc.sync.dma_start(out=bqk_sb, in_=bqk.rearrange("a p -> p a"))
        bv_sb = res.tile([1, DPC], BF16, name="bv_sb")

        loads = []
        for kt in range(NKT):
            loads += [
                (wq_sb[:, kt, :], kt_slice(wq, kt)),
                (xk[kt][:, 0:512], kt_slice(xT_b, kt)[:, 0:512]),
                (wk_sb[:, kt, :], kt_slice(wk, kt)),
            ]
        for kt in range(NKT):
            loads += [
                (xk[kt][:, 512:1024], kt_slice(xT_b, kt)[:, 512:1024]),
                (wqkhi_sb[:, kt, :], kt_slice(wqk_hi, kt)),
            ]
        for kt in range(NKT):
            loads += [
                (xk[kt][:, 1024:1536], kt_slice(xT_b, kt)[:, 1024:1536]),
                (wv_sb[:, kt, :], kt_slice(wv, kt)),
            ]
        for kt in range(NKT):
            loads.append((xk[kt][:, 1536:2048], kt_slice(xT_b, kt)[:, 1536:2048]))
        loads.append((bv_sb, bv[0:1, :]))
        # sync/gpsimd take most traffic (cheap issue); scalar helps at
        # startup (its engine is idle until the first projection lands)
        qpat = [nc.sync, nc.gpsimd, nc.scalar]
        for i, (dst, src) in enumerate(loads):
            qpat[i % len(qpat)].dma_start(out=dst, in_=src)

        # head-stacked Wo slices: heads 0+1 contiguous on partitions 0-127
        # (one contraction-128 matmul), head 2 separate on partitions 0-63.
        woT01_sb = res.tile([128, D], BF16, name="woT01_sb")
        nc.sync.dma_start(out=woT01_sb, in_=woT[0:128, :])
        woT2_sb = res.tile([64, D], BF16, name="woT2_sb")
        nc.gpsimd.dma_start(out=woT2_sb, in_=woT[128:192, :])

        ones_row = res.tile([1, T], BF16, name="ones_row")
        nc.vector.memset(ones_row, 1.0)

        ones_col = res.tile([1, HD], BF16)
        nc.vector.memset(ones_col, 1.0)

        # per-stripe mask mega-tile (128, NJT, 512): one paired DMA per two
        # adjacent key tiles (256 DRAM rows -> (128, 2, 512) strided)
        m_stripes = {}  # ith -> (128, NJT, 512) tile

        def prefetch_masks(ith, queues=(None,)):
            if ith >= NIT:
                return
            isl = slice(ith * IT, (ith + 1) * IT)
            m = mpool.tile([JT, NJT, IT], BF16, tag="mask", name="m_stripe")
            for jp in range(NJT // 2):
                src = maskT[2 * jp * JT : (2 * jp + 2) * JT, isl].rearrange(
                    "(a p) q -> p a q", p=JT
                )
                q = queues[jp % len(queues)] or nc.sync
                q.dma_start(out=m[:, 2 * jp : 2 * jp + 2, :], in_=src)
            m_stripes[ith] = m

        # stripe-0 masks load during the projections, on the two cheapest
        # queues; later stripes prefetch on sync alone.
        prefetch_masks(0, queues=(nc.sync, nc.gpsimd))

        # ---- stage A: projections ------------------------------------------
        qT_a = res.tile([128, T], BF16)  # q heads 0,1 (dims on partitions)
        kT_a = res.tile([128, T], BF16)
        qT_b = res.tile([128, T], BF16)  # q head 2 + duplicate in rows 64-127
        kT_b = res.tile([128, T], BF16)

        def proj_pass(w_sb, nt, emit_out):
            ns = slice(nt * 1024, (nt + 1) * 1024)
            ps = psA.tile([128, 1024], F32, tag="psA", name="psqk")
            for half in range(2):
                fs = slice(nt * 1024 + half * 512, nt * 1024 + half * 512 + 512)
                hs = slice(half * 512, (half + 1) * 512)
                for kt in range(NKT):
                    nc.tensor.matmul(
                        out=ps[:, hs],
                        lhsT=w_sb[:, kt, 0:128],
                        rhs=xk[kt][:, fs],
                        start=(kt == 0),
                        stop=(kt == NKT - 1),
                    )
            emit_out(ps, ns)

        def emit_q(ps, ns):
            nc.scalar.add(out=qT_a[:, ns], in_=ps, add=bqk_sb[:, 0:1])

        def emit_k(ps, ns):
            nc.scalar.add(out=kT_a[:, ns], in_=ps, add=bqk_sb[:, 1:2])

        def emit_hi(ps, ns):
            nc.scalar.add(out=qT_b[0:64, ns], in_=ps[0:64, :], add=bqk_sb[0:64, 2:3])
            nc.scalar.add(
                out=kT_b[0:64, ns], in_=ps[64:128, :], add=bqk_sb[64:128, 2:3]
            )

        # nt=0 passes first (they only need the x column-halves loaded
        # first), then nt=1; PE never waits on the tail of the x load.
        proj_pass(wq_sb, 0, emit_q)
        proj_pass(wk_sb, 0, emit_k)
        proj_pass(wqkhi_sb, 0, emit_hi)
        proj_pass(wq_sb, 1, emit_q)
        proj_pass(wk_sb, 1, emit_k)
        proj_pass(wqkhi_sb, 1, emit_hi)

        # duplicate head 2 q/k into partitions 64-127 (row-group pairing)
        nc.sync.dma_start(out=qT_b[64:128, :], in_=qT_b[0:64, :])
        nc.gpsimd.dma_start(out=kT_b[64:128, :], in_=kT_b[0:64, :])

        # v natural, packed as [v | 1] per head: (128, NJT, HPC, 65) bf16.
        vaug = res.tile([128, NJT, HPC, HD + 1], BF16)
        nc.vector.memset(vaug, 1.0)

        def emit_vproj(jt):
            js = slice(jt * JT, (jt + 1) * JT)
            ps = psW.tile([128, 1024], F32, tag="psW", name="psv")
            for kt in range(NKT):
                nc.tensor.matmul(
                    out=ps[:, 0:DPC],
                    lhsT=xk[kt][:, js],
                    rhs=wv_sb[:, kt, :],
                    start=(kt == 0),
                    stop=False,
                )
            nc.tensor.matmul(
                out=ps[:, 0:DPC], lhsT=ones_row[:, js], rhs=bv_sb,
                start=False, stop=True,
            )
            nc.vector.tensor_copy(
                out=vaug[:, jt, :, 0:HD],
                in_=ps[:, 0:DPC].rearrange("p (h d) -> p h d", h=HPC),
            )

        emit_vproj(0)
        emit_vproj(1)

        # ---- stage B: attention --------------------------------------------
        # Unit list: per query stripe, 16 phase-1 units (heads 0+1, one jt
        # each) then 8 phase-2 units (head 2, a pair of jts). Each unit is
        # one (128, 1024) score tile = two concurrent 512-free matmuls.
        units = []
        for ith in range(NIT):
            for jt in range(NJT):
                units.append(("p1", ith, jt))
            for jp in range(NJT // 2):
                units.append(("p2", ith, 2 * jp, 2 * jp + 1))

        oT = {}  # (ith, h) -> psum tile
        stash = {}  # (ith, h) -> (raw, dn) tiles

        ucount = [0]

        def emit_scores(u):
            isl = slice(u[1] * IT, (u[1] + 1) * IT)
            ps = psA.tile([128, 1024], F32, tag="psA", name="sT")
            eT = epool.tile([JT, 1024], BF16, tag="e", name="eT")
            eTm = empool.tile([JT, 1024], BF16, tag="em", name="eTm")
            m = m_stripes[u[1]]
            if u[0] == "p1":
                jt = u[2]
                js = slice(jt * JT, (jt + 1) * JT)
                # heads 0 (rows 0-63) and 1 (rows 64-127): concurrent
                nc.tensor.matmul(
                    out=ps[:, 0:512], lhsT=kT_a[0:64, js], rhs=qT_a[0:64, isl],
                    start=True, stop=True,
                )
                nc.tensor.matmul(
                    out=ps[:, 512:1024], lhsT=kT_a[64:128, js],
                    rhs=qT_a[64:128, isl], start=True, stop=True,
                )
                # same key-tile mask for both halves: stride-0 broadcast
                m_in = m[:, jt : jt + 1, :].to_broadcast((JT, 2, IT))
            else:
                jt0, jt1 = u[2], u[3]
                js0 = slice(jt0 * JT, (jt0 + 1) * JT)
                js1 = slice(jt1 * JT, (jt1 + 1) * JT)
                # head 2 vs its partition-64 duplicate: concurrent
                nc.tensor.matmul(
                    out=ps[:, 0:512], lhsT=kT_b[0:64, js0], rhs=qT_b[0:64, isl],
                    start=True, stop=True,
                )
                nc.tensor.matmul(
                    out=ps[:, 512:1024], lhsT=kT_b[64:128, js1],
                    rhs=qT_b[64:128, isl], start=True, stop=True,
                )
                m_in = m[:, jt0 : jt0 + 2, :]
                if u[2] == 0:
                    prefetch_masks(u[1] + 1)  # next stripe, during phase 2
            nc.scalar.activation(
                out=eT, in_=ps, func=mybir.ActivationFunctionType.Exp,
                scale=SCALE,
            )
            # mask multiply: every other unit is split half-to-GpSimd (its
            # slower rate is hidden by the AV lag), the rest one Vector op
            if ucount[0] % 2 == 0:
                if u[0] == "p1":
                    m0 = m1 = m[:, u[2], :]
                else:
                    m0, m1 = m[:, u[2], :], m[:, u[3], :]
                nc.vector.tensor_mul(
                    out=eTm[:, 0:512], in0=eT[:, 0:512], in1=m0
                )
                nc.gpsimd.tensor_mul(
                    out=eTm[:, 512:1024], in0=eT[:, 512:1024], in1=m1
                )
            elif u[0] == "p1":
                nc.vector.tensor_mul(
                    out=eTm.rearrange("p (a b) -> p a b", a=2),
                    in0=eT.rearrange("p (a b) -> p a b", a=2),
                    in1=m_in,
                )
            else:
                nc.vector.tensor_mul(
                    out=eTm, in0=eT,
                    in1=m_in.rearrange("p a q -> p (a q)"),
                )
            ucount[0] += 1
            return eTm

        def get_oT(ith, h):
            if (ith, h) not in oT:
                oT[(ith, h)] = psO.tile([128, IT], F32, tag="psO", name="oT")
            return oT[(ith, h)]

        def emit_av_one(u, eTm, h, hs):
            # one AV link: head h of unit u from half hs of its eTm
            jt = u[2] if (u[0] == "p1" or hs.start == 0) else u[3]
            t = get_oT(u[1], h)
            nc.tensor.matmul(
                out=t[0 : HD + 1, :],
                lhsT=vaug[:, jt, h, :],
                rhs=eTm[:, hs],
                start=(jt == 0),
                stop=(jt == NJT - 1),
            )

        H0, H1 = slice(0, 512), slice(512, 1024)

        def emit_av_group(group):
            # h-major across the group so consecutive AV matmuls accumulate
            # into the SAME psum tile (no output-bank alternation)
            if group[0][1][0] == "p1":
                for h, hs in ((0, H0), (1, H1)):
                    for _, u, eTm in group:
                        emit_av_one(u, eTm, h, hs)
            else:
                for _, u, eTm in group:
                    emit_av_one(u, eTm, 2, H0)
                    emit_av_one(u, eTm, 2, H1)

        osb01_all = {}  # ith -> (128, IT) f32r stacked heads 0+1
        osb2_all = {}  # ith -> (64, IT) f32r head 2

        def emit_normalize(ith, h, pool=None):
            # broadcast the denominator to HD partitions on the PE, then
            # reciprocal + scale (both reading across the full partition set)
            raw, dnb = stash.pop((ith, h))
            rb = (pool or psW).tile(
                [128, 1024], F32, tag="psA" if pool is not None else "psW",
                name="rb",
            )
            nc.tensor.matmul(
                out=rb[0:HD, 0:IT], lhsT=ones_col, rhs=dnb,
                start=True, stop=True,
            )
            rcb = small.tile([HD, IT], F32, tag="rcb", name="rcb")
            nc.vector.reciprocal_approx_fast(out=rcb, in_=rb[0:HD, 0:IT])
            if h < 2:
                if ith not in osb01_all:
                    osb01_all[ith] = osbpool.tile(
                        [128, IT], BF16, tag="osb01", name=f"osb01_{ith}"
                    )
                dst = osb01_all[ith][h * HD : (h + 1) * HD, :]
            else:
                osb2_all[ith] = osbpool.tile(
                    [HD, IT], BF16, tag="osb2", name=f"osb2_{ith}"
                )
                dst = osb2_all[ith]
            nc.vector.tensor_mul(out=dst, in0=raw[0:HD, :], in1=rcb)

        def emit_wo_p1(ith, tb, pool=None):
            # heads 0+1 of one 128-token block: one contraction-128 matmul
            # per free chunk into a held psum tile
            ts = slice(tb * 128, (tb + 1) * 128)
            yps = (pool or psW).tile(
                [128, 1024], F32, tag="psA" if pool is not None else "psW",
                name="yps",
            )
            for n0, nsz in ((0, 512), (512, 256)):
                nsl = slice(n0, n0 + nsz)
                nc.tensor.matmul(
                    out=yps[:, nsl], lhsT=osb01_all[ith][:, ts],
                    rhs=woT01_sb[:, nsl], start=True, stop=False,
                )
            return yps

        def emit_wo_p2(ith, tb, yps, yq, cast_eng=None):
            # head 2 accumulates on top, then cast + store
            t0 = ith * IT + tb * 128
            ts = slice(tb * 128, (tb + 1) * 128)
            for n0, nsz in ((0, 512), (512, 256)):
                nsl = slice(n0, n0 + nsz)
                nc.tensor.matmul(
                    out=yps[:, nsl], lhsT=osb2_all[ith][:, ts],
                    rhs=woT2_sb[:, nsl], start=False, stop=True,
                )
            ysb = ypool.tile([128, D], F16, tag="ysb", name="ysb")
            if cast_eng is nc.scalar:
                nc.scalar.copy(out=ysb, in_=yps[:, 0:D])
            else:
                nc.vector.tensor_copy(out=ysb, in_=yps[:, 0:D])
            yq.dma_start(out=y[t0 : t0 + 128, :], in_=ysb)

        def emit_wo(ith, tb, yq, pool=None, cast_eng=None):
            yps = emit_wo_p1(ith, tb, pool=pool)
            emit_wo_p2(ith, tb, yps, yq, cast_eng=cast_eng)

        def emit_stash(u):
            ith = u[1]
            done = []
            if u[0] == "p1" and u[2] == NJT - 1:
                done = [0, 1]
            elif u[0] == "p2" and u[3] == NJT - 1:
                done = [2]
            for h in done:
                t = oT.pop((ith, h))
                dnb = dnpool.tile([1, IT], BF16, tag="dn", name="dnb")
                nc.vector.tensor_copy(out=dnb, in_=t[HD : HD + 1, :])
                raw = rawpool.tile([HD, IT], BF16, tag="raw", name="raw")
                nc.vector.tensor_copy(out=raw, in_=t[0:HD, :])
                stash[(ith, h)] = (raw, dnb)

        # Software-pipelined emission: AV lags scores by 2 units, the
        # previous stripe's normalize + Wo spread through this stripe's
        # units, and the last stripe's h0/h1 normalizes land in its own
        # phase-2 units so the drain is minimal.
        UPS = NJT + NJT // 2  # units per stripe
        LAG = 3

        wo_done = set()

        def do_norm(s, h):
            emit_normalize(s, h)

        def do_wo(s, tb):
            wo_done.add((s, tb))
            emit_wo(s, tb, nc.sync)

        def interleave_tail(idx):
            s, pos = idx // UPS, idx % UPS
            if s < 1:
                return
            if pos == 4:
                do_norm(s - 1, 0)
            elif pos == 7:
                do_norm(s - 1, 1)
            elif pos == 10:
                do_norm(s - 1, 2)
            elif pos in (13, 16, 19, 22):
                do_wo(s - 1, (pos - 13) // 3)
            if s == NIT - 1:
                if pos == 15:
                    do_norm(s, 0)
                elif pos == 17:
                    do_norm(s, 1)

        def pop_avs(n):
            group = [pend.pop(0) for _ in range(n)]
            emit_av_group(group)
            for pi, pu, _ in group:
                emit_stash(pu)
                interleave_tail(pi)

        pend = []
        for i, u in enumerate(units):
            eTm = emit_scores(u)
            if i < NJT - 2:
                emit_vproj(i + 2)
            pend.append((i, u, eTm))
            if len(pend) > LAG:
                if pend[0][1][0] == pend[1][1][0]:
                    pop_avs(2)
                else:
                    pop_avs(1)
        while pend:
            n = 2 if len(pend) >= 2 and pend[0][1][0] == pend[1][1][0] else 1
            pop_avs(n)
        # drain: last stripe's h2 normalize + its 4 output-projection
        # blocks, using the now-free psA banks for 2-deep pipelining and
        # the now-idle ScalarE for half the output casts.
        for ith in range(NIT):
            for h in range(HPC):
                if (ith, h) in stash:
                    emit_normalize(ith, h, pool=psA)
        yqs = [nc.sync, nc.gpsimd, nc.scalar]
        ti = 0
        for ith in range(NIT):
            for tb in range(IT // 128):
                if (ith, tb) not in wo_done:
                    emit_wo(
                        ith, tb, yqs[ti % len(yqs)], pool=psA,
                        cast_eng=nc.scalar if ti % 2 == 0 else nc.vector,
                    )
                    ti += 1


def _host_prep(x, Wq, bq, Wk, bk, Wv, bv, Wo, bo, mask):
    """Build the 8 per-core input maps."""
    x = np.asarray(x, dtype=np.float32)
    mask_np = np.asarray(mask)
    maskT_bf = np.ascontiguousarray(mask_np.T).astype(ml_dtypes.bfloat16)

    xTs = [np.ascontiguousarray(x[b].T).astype(ml_dtypes.bfloat16) for b in range(B)]

    def w_cols(W, cols):
        return np.ascontiguousarray(
            np.asarray(W, np.float32).T[:, cols]
        ).astype(ml_dtypes.bfloat16)

    def wo_rows(W, cols):
        return np.ascontiguousarray(
            np.asarray(W, np.float32).T[cols, :]
        ).astype(ml_dtypes.bfloat16)

    in_maps = []
    for core in range(NCORES):
        b = core // 4
        h0 = HPC * (core % 4)
        cols = np.arange(h0 * HD, (h0 + HPC) * HD)
        wq_a = w_cols(Wq, cols)
        wk_a = w_cols(Wk, cols)
        in_maps.append(
            {
                "xT_b": xTs[b],
                "wq": wq_a,
                "wk": wk_a,
                "wqk_hi": np.concatenate(
                    [wq_a[:, 128:192], wk_a[:, 128:192]], axis=1
                ),
                "bqk": np.stack(
                    [
                        np.asarray(bq, np.float32)[cols][0:128],
                        np.asarray(bk, np.float32)[cols][0:128],
                        np.concatenate(
                            [
                                np.asarray(bq, np.float32)[cols][128:192],
                                np.asarray(bk, np.float32)[cols][128:192],
                            ]
                        ),
                    ]
                ),
                "wv": w_cols(Wv, cols),
                "bv": np.asarray(bv, np.float32)[cols][None, :].astype(
                    ml_dtypes.bfloat16
                ),
                "woT": wo_rows(Wo, cols),
                "maskT": maskT_bf,
            }
        )
    return in_maps


def kernel(x, Wq, bq, Wk, bk, Wv, bv, Wo, bo, mask):
    global _NC, LAST_RESULTS
    if _NC is None:
        _NC = _build_nc()

    in_maps = _host_prep(x, Wq, bq, Wk, bk, Wv, bv, Wo, bo, mask)
    res = run_bass_kernel_spmd(_NC, in_maps, list(range(NCORES)))
    LAST_RESULTS = res

    bo = np.asarray(bo, np.float32)
    out = np.zeros((B, T, D), np.float32)
    for core in range(NCORES):
        out[core // 4] += np.asarray(res.results[core]["y"], np.float32)
    out += bo
    return out


# revision 46
# speedup vs baseline: 1.0106x; 1.0017x over previous
"""BigBird attention on 8 Trainium2 NeuronCores.

Sharding: cores 0-3 take batch 0, cores 4-7 batch 1; each core computes 3 of
the 12 heads end-to-end (q/k/v projection, masked dense attention, its slice
of the output projection). Host work is limited to input transposes/slices
and the final 4-way partial-sum (f16 partials) + output bias.

Per-core dataflow (all matmuls bf16/f32r with f32 PSUM accumulation):
  - Scores are transposed, sT[j, i] = k_j . q_i, contraction depth 64. Heads
    0 and 1 live on partitions 0-63 / 64-127 of qT/kT, so their score
    matmuls occupy disjoint PE row-groups (row tiling); head 2 is duplicated
    into partitions 64-127 and paired across key tiles. Each pair lands in
    one (128, 1024) 2-bank PSUM tile -> one exp activation covers both.
  - exp on ScalarE (scale folded); the mask is applied as ONE bf16
    tensor_tensor multiply per unit (stride-0 broadcast of the key-tile mask
    for phase-1 units, contiguous key-tile-pair slice for phase-2), mostly
    on VectorE with GpSimd taking every 6th unit.
  - AV accumulates [v | 1] so the softmax denominator rides along as row 64.
  - Emission is software-pipelined: AV of unit u is emitted after scores of
    unit u+2 (psA bufs=2), with the previous stripe's normalize + output
    projection spread through this stripe's units.
  - Output projection is head-stacked: heads 0+1 occupy partitions 0-127 of
    one osb tile so their Wo contribution is a single contraction-128
    matmul; head 2 accumulates on top (4 matmuls per 128-token block).
  - The last stripe's h0/h1 normalizes interleave into its phase-2 units so
    the drain is only: h2 normalize + 4 output-projection blocks, with the
    final y DMAs spread across four queues.
"""

import sys

sys.path.insert(0, "/opt/trn_rl_repo")

import numpy as np
import ml_dtypes

import concourse.bass as bass
import concourse.tile as tile
from concourse import bacc
from concourse import mybir
from concourse.bass_utils import run_bass_kernel_spmd

B, T, D, H, HD = 2, 2048, 768, 12, 64
NCORES = 8
HPC = 3  # heads per core
DPC = HPC * HD  # 192 projected dims per core
NKT = D // 128  # 6 contraction tiles (biases handled separately)
SCALE = HD ** -0.5
IT = 512  # query stripe
NIT = T // IT
JT = 128  # key tile (partition dim of transposed scores)
NJT = T // JT

F32 = mybir.dt.float32
F32R = mybir.dt.float32r
BF16 = mybir.dt.bfloat16
F16 = mybir.dt.float16

LAST_RESULTS = None  # BassKernelResults of the most recent run (for test.py)

_NC = None


def _build_nc():
    nc = bacc.Bacc(None, target_bir_lowering=False)

    xT_b = nc.declare_dram_parameter("xT_b", (D, T), BF16, isOutput=False)
    wq = nc.declare_dram_parameter("wq", (D, DPC), BF16, isOutput=False)
    wk = nc.declare_dram_parameter("wk", (D, DPC), BF16, isOutput=False)
    wqk_hi = nc.declare_dram_parameter("wqk_hi", (D, 128), BF16, isOutput=False)
    bqk = nc.declare_dram_parameter("bqk", (3, 128), F32, isOutput=False)
    wv = nc.declare_dram_parameter("wv", (D, DPC), BF16, isOutput=False)
    bv = nc.declare_dram_parameter("bv", (1, DPC), BF16, isOutput=False)
    woT = nc.declare_dram_parameter("woT", (DPC, D), BF16, isOutput=False)
    maskT = nc.declare_dram_parameter("maskT", (T, T), BF16, isOutput=False)
    y = nc.declare_dram_parameter("y", (T, D), F16, isOutput=True)

    with tile.TileContext(nc) as tc:
        _emit(nc, tc, xT_b, wq, wk, wqk_hi, bqk, wv, bv, woT, maskT, y)
    nc.finalize()
    return nc


def _emit(nc, tc, xT_b, wq, wk, wqk_hi, bqk, wv, bv, woT, maskT, y):
    import contextlib

    ctx = contextlib.ExitStack()
    with ctx:
        res = ctx.enter_context(tc.tile_pool(name="res", bufs=1))  # residents
        mpool = ctx.enter_context(tc.tile_pool(name="mask", bufs=2))
        epool = ctx.enter_context(tc.tile_pool(name="e", bufs=14))
        empool = ctx.enter_context(tc.tile_pool(name="em", bufs=14))
        rawpool = ctx.enter_context(tc.tile_pool(name="rawp", bufs=4))
        dnpool = ctx.enter_context(tc.tile_pool(name="dnp", bufs=4))
        osbpool = ctx.enter_context(tc.tile_pool(name="osbp", bufs=6))
        small = ctx.enter_context(tc.tile_pool(name="small", bufs=3))
        ypool = ctx.enter_context(tc.tile_pool(name="ysb", bufs=3))

        psA = ctx.enter_context(tc.tile_pool(name="psA", bufs=2, space="PSUM"))
        psO = ctx.enter_context(tc.tile_pool(name="psO", bufs=2, space="PSUM"))
        psW = ctx.enter_context(tc.tile_pool(name="psW", bufs=1, space="PSUM"))

        # ---- resident loads, spread across the four DMA-capable queues -----
        # Priority order: what the first projection pass (q nt0, k nt0: x
        # halves 0 + wq + wk) needs goes first; everything else follows.
        xk = [res.tile([128, T], BF16, name=f"xk{kt}") for kt in range(NKT)]
        wq_sb = res.tile([128, NKT, DPC], BF16, name="wq_sb")
        wk_sb = res.tile([128, NKT, DPC], BF16, name="wk_sb")
        wv_sb = res.tile([128, NKT, DPC], BF16, name="wv_sb")
        wqkhi_sb = res.tile([128, NKT, 128], BF16, name="wqkhi_sb")

        def kt_slice(dram, kt):
            return dram[kt * 128 : (kt + 1) * 128, :]

        bqk_sb = res.tile([128, 3], F32, name="bqk_sb")
        nc.sync.dma_start(out=bqk_sb, in_=bqk.rearrange("a p -> p a"))
        bv_sb = res.tile([1, DPC], BF16, name="bv_sb")

        loads = []
        for kt in range(NKT):
            loads += [
                (wq_sb[:, kt, :], kt_slice(wq, kt)),
                (xk[kt][:, 0:512], kt_slice(xT_b, kt)[:, 0:512]),
                (wk_sb[:, kt, :], kt_slice(wk, kt)),
            ]
        for kt in range(NKT):
            loads += [
                (xk[kt][:, 512:1024], kt_slice(xT_b, kt)[:, 512:1024]),
                (wqkhi_sb[:, kt, :], kt_slice(wqk_hi, kt)),
                (wv_sb[:, kt, :], kt_slice(wv, kt)),
            ]
        for kt in range(NKT):
            loads += [
                (xk[kt][:, 1024:1536], kt_slice(xT_b, kt)[:, 1024:1536]),
                (xk[kt][:, 1536:2048], kt_slice(xT_b, kt)[:, 1536:2048]),
            ]
        loads.append((bv_sb, bv[0:1, :]))
        # sync/gpsimd take most traffic (cheap issue); scalar helps at
        # startup (its engine is idle until the first projection lands)
        qpat = [nc.sync, nc.gpsimd, nc.scalar]
        for i, (dst, src) in enumerate(loads):
            qpat[i % len(qpat)].dma_start(out=dst, in_=src)

        # head-stacked Wo slices: heads 0+1 contiguous on partitions 0-127
        # (one contraction-128 matmul), head 2 separate on partitions 0-63.
        woT01_sb = res.tile([128, D], BF16, name="woT01_sb")
        nc.sync.dma_start(out=woT01_sb, in_=woT[0:128, :])
        woT2_sb = res.tile([64, D], BF16, name="woT2_sb")
        nc.gpsimd.dma_start(out=woT2_sb, in_=woT[128:192, :])

        ones_row = res.tile([1, T], BF16, name="ones_row")
        nc.vector.memset(ones_row, 1.0)

        ones_col = res.tile([1, HD], BF16)
        nc.vector.memset(ones_col, 1.0)

        # per-stripe mask mega-tile (128, NJT, 512): one paired DMA per two
        # adjacent key tiles (256 DRAM rows -> (128, 2, 512) strided)
        m_stripes = {}  # ith -> (128, NJT, 512) tile

        def prefetch_masks(ith, queues=(None,)):
            if ith >= NIT:
                return
            isl = slice(ith * IT, (ith + 1) * IT)
            m = mpool.tile([JT, NJT, IT], BF16, tag="mask", name="m_stripe")
            for jp in range(NJT // 2):
                src = maskT[2 * jp * JT : (2 * jp + 2) * JT, isl].rearrange(
                    "(a p) q -> p a q", p=JT
                )
                q = queues[jp % len(queues)] or nc.sync
                q.dma_start(out=m[:, 2 * jp : 2 * jp + 2, :], in_=src)
            m_stripes[ith] = m

        # stripe-0 masks load during the projections, on the two cheapest
        # queues; later stripes prefetch on sync alone.
        prefetch_masks(0, queues=(nc.sync, nc.gpsimd))

        # ---- stage A: projections ------------------------------------------
        qT_a = res.tile([128, T], BF16)  # q heads 0,1 (dims on partitions)
        kT_a = res.tile([128, T], BF16)
        qT_b = res.tile([128, T], BF16)  # q head 2 + duplicate in rows 64-127
        kT_b = res.tile([128, T], BF16)

        def proj_pass(w_sb, nt, emit_out):
            ns = slice(nt * 1024, (nt + 1) * 1024)
            ps = psA.tile([128, 1024], F32, tag="psA", name="psqk")
            for half in range(2):
                fs = slice(nt * 1024 + half * 512, nt * 1024 + half * 512 + 512)
                hs = slice(half * 512, (half + 1) * 512)
                for kt in range(NKT):
                    nc.tensor.matmul(
                        out=ps[:, hs],
                        lhsT=w_sb[:, kt, 0:128],
                        rhs=xk[kt][:, fs],
                        start=(kt == 0),
                        stop=(kt == NKT - 1),
                    )
            emit_out(ps, ns)

        def emit_q(ps, ns):
            nc.scalar.add(out=qT_a[:, ns], in_=ps, add=bqk_sb[:, 0:1])

        def emit_k(ps, ns):
            nc.scalar.add(out=kT_a[:, ns], in_=ps, add=bqk_sb[:, 1:2])

        def emit_hi(ps, ns):
            nc.scalar.add(out=qT_b[0:64, ns], in_=ps[0:64, :], add=bqk_sb[0:64, 2:3])
            nc.scalar.add(
                out=kT_b[0:64, ns], in_=ps[64:128, :], add=bqk_sb[64:128, 2:3]
            )

        # Only the q/k nt=0 passes run before attention starts: stripe-0
        # phase-1 units need just qT[:, 0:512] and kT[:, 0:1024], so the
        # remaining projection passes interleave into the first units and
        # ScalarE starts the exp stream ~20us earlier. q nt=1 (queries
        # 1024-2047) is deferred all the way to stripe-0 phase 2.
        proj_pass(wq_sb, 0, emit_q)
        proj_pass(wk_sb, 0, emit_k)

        def _dup_qkb():
            # duplicate head 2 q/k into partitions 64-127 (row-group pairing)
            nc.sync.dma_start(out=qT_b[64:128, :], in_=qT_b[0:64, :])
            nc.gpsimd.dma_start(out=kT_b[64:128, :], in_=kT_b[0:64, :])

        # v natural, packed as [v | 1] per head: (128, NJT, HPC, 65) bf16.
        vaug = res.tile([128, NJT, HPC, HD + 1], BF16)
        nc.vector.memset(vaug, 1.0)

        def emit_vproj(jt):
            js = slice(jt * JT, (jt + 1) * JT)
            ps = psW.tile([128, 1024], F32, tag="psW", name="psv")
            for kt in range(NKT):
                nc.tensor.matmul(
                    out=ps[:, 0:DPC],
                    lhsT=xk[kt][:, js],
                    rhs=wv_sb[:, kt, :],
                    start=(kt == 0),
                    stop=False,
                )
            nc.tensor.matmul(
                out=ps[:, 0:DPC], lhsT=ones_row[:, js], rhs=bv_sb,
                start=False, stop=True,
            )
            nc.vector.tensor_copy(
                out=vaug[:, jt, :, 0:HD],
                in_=ps[:, 0:DPC].rearrange("p (h d) -> p h d", h=HPC),
            )

        # ---- stage B: attention --------------------------------------------
        # Unit list: per query stripe, 16 phase-1 units (heads 0+1, one jt
        # each) then 8 phase-2 units (head 2, a pair of jts). Each unit is
        # one (128, 1024) score tile = two concurrent 512-free matmuls.
        units = []
        for ith in range(NIT):
            for jt in range(NJT):
                units.append(("p1", ith, jt))
            for jp in range(NJT // 2):
                units.append(("p2", ith, 2 * jp, 2 * jp + 1))

        oT = {}  # (ith, h) -> psum tile
        stash = {}  # (ith, h) -> (raw, dn) tiles

        ucount = [0]

        def emit_scores(u):
            isl = slice(u[1] * IT, (u[1] + 1) * IT)
            ps = psA.tile([128, 1024], F32, tag="psA", name="sT")
            eT = epool.tile([JT, 1024], BF16, tag="e", name="eT")
            eTm = empool.tile([JT, 1024], BF16, tag="em", name="eTm")
            m = m_stripes[u[1]]
            if u[0] == "p1":
                jt = u[2]
                js = slice(jt * JT, (jt + 1) * JT)
                # heads 0 (rows 0-63) and 1 (rows 64-127): concurrent
                nc.tensor.matmul(
                    out=ps[:, 0:512], lhsT=kT_a[0:64, js], rhs=qT_a[0:64, isl],
                    start=True, stop=True,
                )
                nc.tensor.matmul(
                    out=ps[:, 512:1024], lhsT=kT_a[64:128, js],
                    rhs=qT_a[64:128, isl], start=True, stop=True,
                )
                # same key-tile mask for both halves: stride-0 broadcast
                m_in = m[:, jt : jt + 1, :].to_broadcast((JT, 2, IT))
            else:
                jt0, jt1 = u[2], u[3]
                js0 = slice(jt0 * JT, (jt0 + 1) * JT)
                js1 = slice(jt1 * JT, (jt1 + 1) * JT)
                # head 2 vs its partition-64 duplicate: concurrent
                nc.tensor.matmul(
                    out=ps[:, 0:512], lhsT=kT_b[0:64, js0], rhs=qT_b[0:64, isl],
                    start=True, stop=True,
                )
                nc.tensor.matmul(
                    out=ps[:, 512:1024], lhsT=kT_b[64:128, js1],
                    rhs=qT_b[64:128, isl], start=True, stop=True,
                )
                m_in = m[:, jt0 : jt0 + 2, :]
                if u[2] == 0:
                    prefetch_masks(u[1] + 1)  # next stripe, during phase 2
            nc.scalar.activation(
                out=eT, in_=ps, func=mybir.ActivationFunctionType.Exp,
                scale=SCALE,
            )
            # mask multiply: every other unit is split half-to-GpSimd (its
            # slower rate is hidden by the AV lag), the rest one Vector op
            if ucount[0] % 2 == 0:
                if u[0] == "p1":
                    m0 = m1 = m[:, u[2], :]
                else:
                    m0, m1 = m[:, u[2], :], m[:, u[3], :]
                nc.vector.tensor_mul(
                    out=eTm[:, 0:512], in0=eT[:, 0:512], in1=m0
                )
                nc.gpsimd.tensor_mul(
                    out=eTm[:, 512:1024], in0=eT[:, 512:1024], in1=m1
                )
            elif u[0] == "p1":
                nc.vector.tensor_mul(
                    out=eTm.rearrange("p (a b) -> p a b", a=2),
                    in0=eT.rearrange("p (a b) -> p a b", a=2),
                    in1=m_in,
                )
            else:
                nc.vector.tensor_mul(
                    out=eTm, in0=eT,
                    in1=m_in.rearrange("p a q -> p (a q)"),
                )
            ucount[0] += 1
            return eTm

        def get_oT(ith, h):
            if (ith, h) not in oT:
                oT[(ith, h)] = psO.tile([128, IT], F32, tag="psO", name="oT")
            return oT[(ith, h)]

        def emit_av_one(u, eTm, h, hs):
            # one AV link: head h of unit u from half hs of its eTm
            jt = u[2] if (u[0] == "p1" or hs.start == 0) else u[3]
            t = get_oT(u[1], h)
            nc.tensor.matmul(
                out=t[0 : HD + 1, :],
                lhsT=vaug[:, jt, h, :],
                rhs=eTm[:, hs],
                start=(jt == 0),
                stop=(jt == NJT - 1),
            )

        H0, H1 = slice(0, 512), slice(512, 1024)

        def emit_av_group(group):
            # h-major across the group so consecutive AV matmuls accumulate
            # into the SAME psum tile (no output-bank alternation)
            if group[0][1][0] == "p1":
                for h, hs in ((0, H0), (1, H1)):
                    for _, u, eTm in group:
                        emit_av_one(u, eTm, h, hs)
            else:
                for _, u, eTm in group:
                    emit_av_one(u, eTm, 2, H0)
                    emit_av_one(u, eTm, 2, H1)

        osb01_all = {}  # ith -> (128, IT) f32r stacked heads 0+1
        osb2_all = {}  # ith -> (64, IT) f32r head 2

        def emit_normalize(ith, h, pool=None):
            # broadcast the denominator to HD partitions on the PE, then
            # reciprocal + scale (both reading across the full partition set)
            raw, dnb = stash.pop((ith, h))
            rb = (pool or psW).tile(
                [128, 1024], F32, tag="psA" if pool is not None else "psW",
                name="rb",
            )
            nc.tensor.matmul(
                out=rb[0:HD, 0:IT], lhsT=ones_col, rhs=dnb,
                start=True, stop=True,
            )
            rcb = small.tile([HD, IT], F32, tag="rcb", name="rcb")
            nc.vector.reciprocal_approx_fast(out=rcb, in_=rb[0:HD, 0:IT])
            if h < 2:
                if ith not in osb01_all:
                    osb01_all[ith] = osbpool.tile(
                        [128, IT], BF16, tag="osb01", name=f"osb01_{ith}"
                    )
                dst = osb01_all[ith][h * HD : (h + 1) * HD, :]
            else:
                osb2_all[ith] = osbpool.tile(
                    [HD, IT], BF16, tag="osb2", name=f"osb2_{ith}"
                )
                dst = osb2_all[ith]
            nc.vector.tensor_mul(out=dst, in0=raw[0:HD, :], in1=rcb)

        def emit_wo_p1(ith, tb, pool=None):
            # heads 0+1 of one 128-token block: one contraction-128 matmul
            # per free chunk into a held psum tile
            ts = slice(tb * 128, (tb + 1) * 128)
            yps = (pool or psW).tile(
                [128, 1024], F32, tag="psA" if pool is not None else "psW",
                name="yps",
            )
            for n0, nsz in ((0, 512), (512, 256)):
                nsl = slice(n0, n0 + nsz)
                nc.tensor.matmul(
                    out=yps[:, nsl], lhsT=osb01_all[ith][:, ts],
                    rhs=woT01_sb[:, nsl], start=True, stop=False,
                )
            return yps

        def emit_wo_p2(ith, tb, yps, yq, cast_eng=None):
            # head 2 accumulates on top, then cast + store
            t0 = ith * IT + tb * 128
            ts = slice(tb * 128, (tb + 1) * 128)
            for n0, nsz in ((0, 512), (512, 256)):
                nsl = slice(n0, n0 + nsz)
                nc.tensor.matmul(
                    out=yps[:, nsl], lhsT=osb2_all[ith][:, ts],
                    rhs=woT2_sb[:, nsl], start=False, stop=True,
                )
            ysb = ypool.tile([128, D], F16, tag="ysb", name="ysb")
            if cast_eng is nc.scalar:
                nc.scalar.copy(out=ysb, in_=yps[:, 0:D])
            else:
                nc.vector.tensor_copy(out=ysb, in_=yps[:, 0:D])
            yq.dma_start(out=y[t0 : t0 + 128, :], in_=ysb)

        def emit_wo(ith, tb, yq, pool=None, cast_eng=None):
            yps = emit_wo_p1(ith, tb, pool=pool)
            emit_wo_p2(ith, tb, yps, yq, cast_eng=cast_eng)

        def emit_stash(u):
            ith = u[1]
            done = []
            if u[0] == "p1" and u[2] == NJT - 1:
                done = [0, 1]
            elif u[0] == "p2" and u[3] == NJT - 1:
                done = [2]
            for h in done:
                t = oT.pop((ith, h))
                dnb = dnpool.tile([1, IT], BF16, tag="dn", name="dnb")
                nc.vector.tensor_copy(out=dnb, in_=t[HD : HD + 1, :])
                raw = rawpool.tile([HD, IT], BF16, tag="raw", name="raw")
                nc.vector.tensor_copy(out=raw, in_=t[0:HD, :])
                stash[(ith, h)] = (raw, dnb)

        # Software-pipelined emission: AV lags scores by 2 units, the
        # previous stripe's normalize + Wo spread through this stripe's
        # units, and the last stripe's h0/h1 normalizes land in its own
        # phase-2 units so the drain is minimal.
        UPS = NJT + NJT // 2  # units per stripe
        LAG = 3

        wo_done = set()

        def do_norm(s, h):
            emit_normalize(s, h)

        def do_wo(s, tb):
            wo_done.add((s, tb))
            emit_wo(s, tb, nc.sync)

        def interleave_tail(idx):
            s, pos = idx // UPS, idx % UPS
            if s < 1:
                return
            if pos == 4:
                do_norm(s - 1, 0)
            elif pos == 7:
                do_norm(s - 1, 1)
            elif pos == 10:
                do_norm(s - 1, 2)
            elif pos in (13, 16, 19, 22):
                do_wo(s - 1, (pos - 13) // 3)
            if s == NIT - 1:
                if pos == 15:
                    do_norm(s, 0)
                elif pos == 17:
                    do_norm(s, 1)

        def pop_avs(n):
            group = [pend.pop(0) for _ in range(n)]
            emit_av_group(group)
            for pi, pu, _ in group:
                emit_stash(pu)
                interleave_tail(pi)

        # deferred startup work, keyed by unit index (runs before that
        # unit's scores): remaining projections + dup + v projections
        hooks = {
            2: [lambda: proj_pass(wqkhi_sb, 0, emit_hi)],
            4: [lambda: proj_pass(wk_sb, 1, emit_k)],
            6: [lambda: proj_pass(wqkhi_sb, 1, emit_hi), _dup_qkb,
                lambda: emit_vproj(0), lambda: emit_vproj(1)],
            7: [lambda: emit_vproj(2), lambda: emit_vproj(3)],
            8: [lambda: emit_vproj(4), lambda: emit_vproj(5)],
            9: [lambda: emit_vproj(6), lambda: emit_vproj(7)],
            20: [lambda: proj_pass(wq_sb, 1, emit_q)],
        }
        for jt in range(8, NJT):
            hooks.setdefault(jt + 2, []).append(
                lambda jt=jt: emit_vproj(jt)
            )
        POPSTART = 12  # let the AV backlog build while projections finish

        pend = []
        for i, u in enumerate(units):
            for h in hooks.get(i, ()):
                h()
            eTm = emit_scores(u)
            pend.append((i, u, eTm))
            if i >= POPSTART and len(pend) > LAG:
                if pend[0][1][0] == pend[1][1][0]:
                    pop_avs(2)
                else:
                    pop_avs(1)
        while pend:
            n = 2 if len(pend) >= 2 and pend[0][1][0] == pend[1][1][0] else 1
            pop_avs(n)
        # drain: last stripe's h2 normalize + its 4 output-projection
        # blocks, using the now-free psA banks for 2-deep pipelining and
        # the now-idle ScalarE for half the output casts.
        for ith in range(NIT):
            for h in range(HPC):
                if (ith, h) in stash:
                    emit_normalize(ith, h, pool=psA)
        yqs = [nc.sync, nc.gpsimd, nc.scalar]
        ti = 0
        for ith in range(NIT):
            for tb in range(IT // 128):
                if (ith, tb) not in wo_done:
                    emit_wo(
                        ith, tb, yqs[ti % len(yqs)], pool=psA,
                        cast_eng=nc.scalar if ti % 2 == 0 else nc.vector,
                    )
                    ti += 1


def _host_prep(x, Wq, bq, Wk, bk, Wv, bv, Wo, bo, mask):
    """Build the 8 per-core input maps."""
    x = np.asarray(x, dtype=np.float32)
    mask_np = np.asarray(mask)
    maskT_bf = np.ascontiguousarray(mask_np.T).astype(ml_dtypes.bfloat16)

    xTs = [np.ascontiguousarray(x[b].T).astype(ml_dtypes.bfloat16) for b in range(B)]

    def w_cols(W, cols):
        return np.ascontiguousarray(
            np.asarray(W, np.float32).T[:, cols]
        ).astype(ml_dtypes.bfloat16)

    def wo_rows(W, cols):
        return np.ascontiguousarray(
            np.asarray(W, np.float32).T[cols, :]
        ).astype(ml_dtypes.bfloat16)

    in_maps = []
    for core in range(NCORES):
        b = core // 4
        h0 = HPC * (core % 4)
        cols = np.arange(h0 * HD, (h0 + HPC) * HD)
        wq_a = w_cols(Wq, cols)
        wk_a = w_cols(Wk, cols)
        in_maps.append(
            {
                "xT_b": xTs[b],
                "wq": wq_a,
                "wk": wk_a,
                "wqk_hi": np.concatenate(
                    [wq_a[:, 128:192], wk_a[:, 128:192]], axis=1
                ),
                "bqk": np.stack(
                    [
                        np.asarray(bq, np.float32)[cols][0:128],
                        np.asarray(bk, np.float32)[cols][0:128],
                        np.concatenate(
                            [
                                np.asarray(bq, np.float32)[cols][128:192],
                                np.asarray(bk, np.float32)[cols][128:192],
                            ]
                        ),
                    ]
                ),
                "wv": w_cols(Wv, cols),
                "bv": np.asarray(bv, np.float32)[cols][None, :].astype(
                    ml_dtypes.bfloat16
                ),
                "woT": wo_rows(Wo, cols),
                "maskT": maskT_bf,
            }
        )
    return in_maps


def kernel(x, Wq, bq, Wk, bk, Wv, bv, Wo, bo, mask):
    global _NC, LAST_RESULTS
    if _NC is None:
        _NC = _build_nc()

    in_maps = _host_prep(x, Wq, bq, Wk, bk, Wv, bv, Wo, bo, mask)
    res = run_bass_kernel_spmd(_NC, in_maps, list(range(NCORES)))
    LAST_RESULTS = res

    bo = np.asarray(bo, np.float32)
    out = np.zeros((B, T, D), np.float32)
    for core in range(NCORES):
        out[core // 4] += np.asarray(res.results[core]["y"], np.float32)
    out += bo
    return out
